# revision 13
# baseline (speedup 1.0000x reference)
"""Block-circulant matvec (FFT linear layer) as dense TensorE matmuls on 8 TRN2 cores.

Math: the reference computes, per output block o,
    y[o, :] = sum_j IFFT(FFT(w[o,j]) * FFT(x[j])).real
which is a sum of circular convolutions:
    y[o, a] = sum_{j, b} w[o, j, b] * x[j, (a - b) mod 128]

Rewritten as matmuls: for each phase b and input-block tile jt (4 tiles of 128),
    YT[a, o] += XR(b,jt)[j', a]^T @ WT(b,jt)[j', o]
where XR(b,jt)[j', a] = x[jt*128+j', (a-b) mod 128] (rotated x tile, stationary)
and   WT(b,jt)[j', o] = w[o, jt*128+j', b]          (moving operand, N=512).

Sharding: the 128 phases b are split 16-per-core across 8 cores; each core
accumulates its 64 (b, jt) groups into one PSUM bank [128a x 512o] and writes a
partial YT. The host sums the 8 partials (no collective needed).

Precision: weights are centered (r = w - 0.5, uniform in [-0.5, 0.5]) and
shipped as fp8 e4m3 — halving HBM traffic vs bf16 (4.2 MiB/core, streamed at
the ~370-430 GB/s DMA floor).  x is also fp8 e4m3, and consecutive group
pairs fuse into DoubleRow matmuls (256-deep contraction, 2 fp8 MACs per cell
per cycle) so the 32 matmuls take only ~7 us and the kernel is DMA-bound.
Measured end-to-end quantization error is 0.87% (gate 2e-2).  The rank-1
part folds into one host-side scalar add after the gather (w = 0.5 + r makes
y = R@x + 0.5*sum(x)*ones).

On-chip details: the stationary operand of each matmul reads its rotated
window DIRECTLY from the tiny doubled-x buffer via an overlapping-window AP
(lhsT[j', t, a] = xb2[j', jt+t, q + a]) -- no materialized rotated copies, so
the Vector engine does no bulk work and the weight DMA stream keeps the SBUF
ports to itself.  The per-core phase offset is folded into a host-side roll
of the x buffer so the SPMD program is core-independent.  All input DMA rides
the sync-engine HWDGE ring in consumption order (x, then uniform 8-group
weight chunks whose ~64% PE duty cycle holds the HAM clock gate at 2.4 GHz);
N=128 warm-up matmuls cover the gate's ~3.4us ramp before the first chunk
lands, and a few filler matmuls after each chunk burst top up PE activity so
the gate never re-throttles mid-stream.  PSUM is evacuated in halves on the
Scalar and Vector engines in parallel, stored via both HWDGE rings.
"""

import numpy as np
import ml_dtypes

O_BLOCKS = 512
I_BLOCKS = 512
BLOCK = 128
N_CORES = 8
B_PER_CORE = BLOCK // N_CORES          # 16 phases per core
JT_TILES = I_BLOCKS // 128             # 4 contraction tiles
N_GROUPS = B_PER_CORE * JT_TILES       # 64 matmul groups per core
XB2W = 144  # doubled-x width: windows read [q, q+127], q<16 -> 143 max
CHUNK_GROUPS = (8, 8, 8, 8, 8, 8, 8, 4, 2, 2)  # all even: DoubleRow pairs never straddle
assert sum(CHUNK_GROUPS) == N_GROUPS
N_WARMUP_MM = 36   # ~3.8us of sustained PE busy: flips the HAM clock gate to
# 2.4 GHz just before the first real chunk's matmuls issue
N_FILLER_MM = 2    # cheap matmuls after each chunk burst keep PE duty high
# enough during DMA waits that the gate never re-throttles

MODE = "dr"  # "dr" (DoubleRow fp8e4) or "mixed" (bf16 x fp8e3)

_BF16 = ml_dtypes.bfloat16
_FP8E3 = ml_dtypes.float8_e3m4
_FP8E4 = ml_dtypes.float8_e4m3

_MODULE_CACHE = {}


def _build_module():
    import concourse.bass as bass
    import concourse.bacc as bacc
    import concourse.mybir as mybir
    from concourse import tile

    dr = MODE == "dr"
    x_dt = mybir.dt.float8e4 if dr else mybir.dt.bfloat16
    w_dt = mybir.dt.float8e4 if dr else mybir.dt.float8e3

    nc = bacc.Bacc(
        "TRN2",
        target_bir_lowering=False,
        debug=False,
        enable_asserts=False,
        enable_partition_id=False,
        num_devices=N_CORES,
    )

    xb2_d = nc.dram_tensor("xb2", [128, JT_TILES, XB2W], x_dt, kind="ExternalInput")
    wt_d = nc.dram_tensor("wt", [128, N_GROUPS, O_BLOCKS], w_dt, kind="ExternalInput")
    yt_d = nc.dram_tensor("yt", [BLOCK, O_BLOCKS], mybir.dt.float32, kind="ExternalOutput")

    with tile.TileContext(nc) as tc:
        with (
            tc.tile_pool(name="xbp", bufs=1) as xbp,
            tc.tile_pool(name="wtp", bufs=len(CHUNK_GROUPS)) as wtp,
            tc.tile_pool(name="psp", bufs=2, space="PSUM") as psp,
            tc.tile_pool(name="outp", bufs=1) as outp,
            tc.tile_pool(name="scrp", bufs=1) as scrp,
        ):
            scr = scrp.tile([128, 2 * BLOCK], mybir.dt.bfloat16)
            nc.gpsimd.memset(scr[:], 0.0)
            ps_warm = psp.tile([BLOCK, BLOCK], mybir.dt.float32)

            def scr_mm(n):
                for _ in range(n):
                    nc.tensor.matmul(
                        ps_warm[:], scr[:, :BLOCK], scr[:, BLOCK:],
                        start=True, stop=True, skip_group_check=True,
                    )

            scr_mm(N_WARMUP_MM)

            # all input DMA on the sync ring, in consumption order
            xb2_sb = xbp.tile([128, JT_TILES, XB2W], x_dt)
            nc.sync.dma_start(xb2_sb[:], xb2_d[:])
            xb2_ap = xb2_sb[:]

            def lhsT_window(m):
                # stationary operand for pair m: groups (2m, 2m+1), i.e.
                # q = m // 2, jt0 = 2*(m % 2); lhsT[p, t, a] = xb2[p, jt0+t, q+a]
                q, jt0 = m // 2, 2 * (m % 2)
                return bass.AP(
                    tensor=xb2_ap.tensor,
                    offset=xb2_ap.offset + jt0 * XB2W + q,
                    ap=[
                        xb2_ap.ap[0],        # partition (j')
                        [XB2W, 2],           # t: jt pair member
                        [1, BLOCK],          # a (overlapping window)
                    ],
                )

            def lhsT_single(g):
                q, jt = divmod(g, JT_TILES)
                return bass.AP(
                    tensor=xb2_ap.tensor,
                    offset=xb2_ap.offset + jt * XB2W + q,
                    ap=[xb2_ap.ap[0], [1, BLOCK]],
                )

            ps = psp.tile([BLOCK, O_BLOCKS], mybir.dt.float32)

            g0 = 0
            for ci, n_g in enumerate(CHUNK_GROUPS):
                wt_sb = wtp.tile([128, n_g, O_BLOCKS], w_dt, tag="wchunk")
                nc.sync.dma_start(wt_sb[:], wt_d[:, g0 : g0 + n_g, :])
                if dr:
                    for pi in range(n_g // 2):
                        g = g0 + 2 * pi
                        nc.tensor.matmul(
                            ps[:],
                            lhsT_window(g // 2),
                            wt_sb[:, 2 * pi : 2 * pi + 2, :],
                            start=(g == 0),
                            stop=(g == N_GROUPS - 2),
                            perf_mode=mybir.MatmulPerfMode.DoubleRow,
                            skip_group_check=True,
                        )
                else:
                    for gi in range(n_g):
                        g = g0 + gi
                        nc.tensor.matmul(
                            ps[:],
                            lhsT_single(g),
                            wt_sb[:, gi, :],
                            start=(g == 0),
                            stop=(g == N_GROUPS - 1),
                            skip_group_check=True,
                        )
                g0 += n_g
                if g0 < N_GROUPS:
                    scr_mm(N_FILLER_MM)

            # evacuate PSUM in quarters, alternating Scalar/Vector copies so
            # both run in parallel and the first store issues after only a
            # quarter-copy; stores alternate between the two HWDGE rings
            out_sb = outp.tile([BLOCK, O_BLOCKS], mybir.dt.float32)
            qo = O_BLOCKS // 4
            for k in range(4):
                lo, hi = k * qo, (k + 1) * qo
                if k % 2 == 0:
                    nc.scalar.copy(out_sb[:, lo:hi], ps[:, lo:hi])
                    nc.scalar.dma_start(yt_d[:, lo:hi], out_sb[:, lo:hi])
                else:
                    nc.vector.tensor_copy(out_sb[:, lo:hi], ps[:, lo:hi])
                    nc.sync.dma_start(yt_d[:, lo:hi], out_sb[:, lo:hi])

    nc.compile()
    return nc


def _get_module():
    if "nc" not in _MODULE_CACHE:
        _MODULE_CACHE["nc"] = _build_module()
    return _MODULE_CACHE["nc"]


def _prepare_inputs(x, cir_weights):
    x_np = _FP8E4 if MODE == "dr" else _BF16
    w_np = _FP8E4 if MODE == "dr" else _FP8E3

    xb = np.asarray(x, dtype=np.float32).reshape(I_BLOCKS, BLOCK)
    W = np.asarray(cir_weights, dtype=np.float32)

    # centered weights r = w - 0.5 in [-0.5, 0.5]; [b, j, o] fp8, contiguous
    WT = np.ascontiguousarray((W - np.float32(0.5)).astype(w_np).transpose(2, 1, 0))

    xx = xb.astype(x_np).reshape(JT_TILES, 128, BLOCK)  # [jt, j', c]

    in_maps = []
    for c in range(N_CORES):
        # Group order on core c: g = q*JT_TILES + jt with phase b = 16c+15-q,
        # so the on-chip window walk (src offset q+a) sees ascending q.
        # Host-side roll D_c makes the fixed kernel offset correct per core:
        #   xb2_c[j', jt, cc] = xb[jt*128+j', (cc + D_c) mod 128]
        D_c = (-(B_PER_CORE * c) - (B_PER_CORE - 1)) % BLOCK
        rolled = np.roll(xx, -D_c, axis=2)               # [jt, j', c]
        xb2 = np.concatenate([rolled, rolled], axis=2)[:, :, :XB2W]  # [jt, j', 144]
        xb2 = np.ascontiguousarray(xb2.transpose(1, 0, 2))  # [j', jt, 256]

        sub = WT[c * B_PER_CORE : (c + 1) * B_PER_CORE]  # [b_idx, j, o], b asc
        sub = sub[::-1]                                  # q = 15 - b_idx
        sub = sub.reshape(N_GROUPS, 128, O_BLOCKS)       # [g=(q,jt), j', o]
        wt = np.ascontiguousarray(sub.transpose(1, 0, 2))  # [j', g, o]

        in_maps.append({"xb2": xb2, "wt": wt})
    return in_maps


def kernel(x, cir_weights):
    from concourse.bass_utils import run_bass_kernel_spmd

    nc = _get_module()
    in_maps = _prepare_inputs(x, cir_weights)
    res = run_bass_kernel_spmd(nc, in_maps, core_ids=list(range(N_CORES)))

    yt = np.zeros((BLOCK, O_BLOCKS), dtype=np.float32)
    for r in res.results:
        yt += r["yt"]
    # rank-1 part of the centered-weight decomposition: w = 0.5 + r makes
    # y = R@x + 0.5*sum(x) in every output coordinate
    yt += np.float32(0.5 * np.sum(np.asarray(x, dtype=np.float64)))
    return np.ascontiguousarray(yt.T).reshape(O_BLOCKS * BLOCK)


# revision 14
# speedup vs baseline: 1.0401x; 1.0401x over previous
"""Block-circulant matvec (FFT linear layer) as dense TensorE matmuls on 8 TRN2 cores.

Math: the reference computes, per output block o,
    y[o, :] = sum_j IFFT(FFT(w[o,j]) * FFT(x[j])).real
which is a sum of circular convolutions:
    y[o, a] = sum_{j, b} w[o, j, b] * x[j, (a - b) mod 128]

Rewritten as matmuls: for each phase b and input-block tile jt (4 tiles of 128),
    YT[a, o] += XR(b,jt)[j', a]^T @ WT(b,jt)[j', o]
where XR(b,jt)[j', a] = x[jt*128+j', (a-b) mod 128] (rotated x tile, stationary)
and   WT(b,jt)[j', o] = w[o, jt*128+j', b]          (moving operand, N=512).

Sharding: the 128 phases b are split 16-per-core across 8 cores; each core
accumulates its 64 (b, jt) groups into one PSUM bank [128a x 512o] and writes a
partial YT. The host sums the 8 partials (no collective needed).

Precision: weights are centered (r = w - 0.5, uniform in [-0.5, 0.5]) and
shipped as fp8 e4m3 — halving HBM traffic vs bf16 (4.2 MiB/core, streamed at
the ~370-430 GB/s DMA floor).  x is also fp8 e4m3, and consecutive group
pairs fuse into DoubleRow matmuls (256-deep contraction, 2 fp8 MACs per cell
per cycle) so the 32 matmuls take only ~7 us and the kernel is DMA-bound.
Measured end-to-end quantization error is 0.87% (gate 2e-2).  The rank-1
part folds into one host-side scalar add after the gather (w = 0.5 + r makes
y = R@x + 0.5*sum(x)*ones).

On-chip details: the stationary operand of each matmul reads its rotated
window DIRECTLY from the tiny doubled-x buffer via an overlapping-window AP
(lhsT[j', t, a] = xb2[j', jt+t, q + a]) -- no materialized rotated copies, so
the Vector engine does no bulk work and the weight DMA stream keeps the SBUF
ports to itself.  The per-core phase offset is folded into a host-side roll
of the x buffer so the SPMD program is core-independent.  All input DMA rides
the sync-engine HWDGE ring in consumption order (x, then uniform 8-group
weight chunks whose ~64% PE duty cycle holds the HAM clock gate at 2.4 GHz);
N=128 warm-up matmuls cover the gate's ~3.4us ramp before the first chunk
lands, and a few filler matmuls after each chunk burst top up PE activity so
the gate never re-throttles mid-stream.  PSUM is evacuated in halves on the
Scalar and Vector engines in parallel, stored via both HWDGE rings.
"""

import numpy as np
import ml_dtypes

O_BLOCKS = 512
I_BLOCKS = 512
BLOCK = 128
N_CORES = 8
B_PER_CORE = BLOCK // N_CORES          # 16 phases per core
JT_TILES = I_BLOCKS // 128             # 4 contraction tiles
N_GROUPS = B_PER_CORE * JT_TILES       # 64 matmul groups per core
XB2W = 144  # doubled-x width: windows read [q, q+127], q<16 -> 143 max
CHUNK_GROUPS = (16, 16, 16, 8, 4, 2, 2)  # all even: DoubleRow pairs never straddle
assert sum(CHUNK_GROUPS) == N_GROUPS
N_WARMUP_MM = 36   # ~3.8us of sustained PE busy: flips the HAM clock gate to
# 2.4 GHz just before the first real chunk's matmuls issue
N_FILLER_MM = 4    # cheap matmuls after each chunk burst keep PE duty high
# enough during DMA waits that the gate never re-throttles

MODE = "dr"  # "dr" (DoubleRow fp8e4) or "mixed" (bf16 x fp8e3)

_BF16 = ml_dtypes.bfloat16
_FP8E3 = ml_dtypes.float8_e3m4
_FP8E4 = ml_dtypes.float8_e4m3

_MODULE_CACHE = {}


def _build_module():
    import concourse.bass as bass
    import concourse.bacc as bacc
    import concourse.mybir as mybir
    from concourse import tile

    dr = MODE == "dr"
    x_dt = mybir.dt.float8e4 if dr else mybir.dt.bfloat16
    w_dt = mybir.dt.float8e4 if dr else mybir.dt.float8e3

    nc = bacc.Bacc(
        "TRN2",
        target_bir_lowering=False,
        debug=False,
        enable_asserts=False,
        enable_partition_id=False,
        num_devices=N_CORES,
    )

    xb2_d = nc.dram_tensor("xb2", [128, JT_TILES, XB2W], x_dt, kind="ExternalInput")
    wt_d = nc.dram_tensor("wt", [128, N_GROUPS, O_BLOCKS], w_dt, kind="ExternalInput")
    yt_d = nc.dram_tensor("yt", [BLOCK, O_BLOCKS], mybir.dt.float32, kind="ExternalOutput")

    with tile.TileContext(nc) as tc:
        with (
            tc.tile_pool(name="xbp", bufs=1) as xbp,
            tc.tile_pool(name="wtp", bufs=len(CHUNK_GROUPS)) as wtp,
            tc.tile_pool(name="psp", bufs=2, space="PSUM") as psp,
            tc.tile_pool(name="outp", bufs=1) as outp,
            tc.tile_pool(name="scrp", bufs=1) as scrp,
        ):
            scr = scrp.tile([128, 2 * BLOCK], mybir.dt.bfloat16)
            nc.gpsimd.memset(scr[:], 0.0)
            ps_warm = psp.tile([BLOCK, BLOCK], mybir.dt.float32)

            def scr_mm(n):
                for _ in range(n):
                    nc.tensor.matmul(
                        ps_warm[:], scr[:, :BLOCK], scr[:, BLOCK:],
                        start=True, stop=True, skip_group_check=True,
                    )

            scr_mm(N_WARMUP_MM)

            # all input DMA on the sync ring, in consumption order
            xb2_sb = xbp.tile([128, JT_TILES, XB2W], x_dt)
            nc.sync.dma_start(xb2_sb[:], xb2_d[:])
            xb2_ap = xb2_sb[:]

            def lhsT_window(m):
                # stationary operand for pair m: groups (2m, 2m+1), i.e.
                # q = m // 2, jt0 = 2*(m % 2); lhsT[p, t, a] = xb2[p, jt0+t, q+a]
                q, jt0 = m // 2, 2 * (m % 2)
                return bass.AP(
                    tensor=xb2_ap.tensor,
                    offset=xb2_ap.offset + jt0 * XB2W + q,
                    ap=[
                        xb2_ap.ap[0],        # partition (j')
                        [XB2W, 2],           # t: jt pair member
                        [1, BLOCK],          # a (overlapping window)
                    ],
                )

            def lhsT_single(g):
                q, jt = divmod(g, JT_TILES)
                return bass.AP(
                    tensor=xb2_ap.tensor,
                    offset=xb2_ap.offset + jt * XB2W + q,
                    ap=[xb2_ap.ap[0], [1, BLOCK]],
                )

            ps = psp.tile([BLOCK, O_BLOCKS], mybir.dt.float32)

            g0 = 0
            for ci, n_g in enumerate(CHUNK_GROUPS):
                wt_sb = wtp.tile([128, n_g, O_BLOCKS], w_dt, tag="wchunk")
                nc.sync.dma_start(wt_sb[:], wt_d[:, g0 : g0 + n_g, :])
                if dr:
                    for pi in range(n_g // 2):
                        g = g0 + 2 * pi
                        nc.tensor.matmul(
                            ps[:],
                            lhsT_window(g // 2),
                            wt_sb[:, 2 * pi : 2 * pi + 2, :],
                            start=(g == 0),
                            stop=(g == N_GROUPS - 2),
                            perf_mode=mybir.MatmulPerfMode.DoubleRow,
                            skip_group_check=True,
                        )
                else:
                    for gi in range(n_g):
                        g = g0 + gi
                        nc.tensor.matmul(
                            ps[:],
                            lhsT_single(g),
                            wt_sb[:, gi, :],
                            start=(g == 0),
                            stop=(g == N_GROUPS - 1),
                            skip_group_check=True,
                        )
                g0 += n_g
                if g0 < N_GROUPS:
                    scr_mm(N_FILLER_MM)

            # evacuate PSUM in quarters, alternating Scalar/Vector copies so
            # both run in parallel and the first store issues after only a
            # quarter-copy; stores alternate between the two HWDGE rings
            out_sb = outp.tile([BLOCK, O_BLOCKS], mybir.dt.float32)
            qo = O_BLOCKS // 4
            for k in range(4):
                lo, hi = k * qo, (k + 1) * qo
                if k % 2 == 0:
                    nc.scalar.copy(out_sb[:, lo:hi], ps[:, lo:hi])
                    nc.scalar.dma_start(yt_d[:, lo:hi], out_sb[:, lo:hi])
                else:
                    nc.vector.tensor_copy(out_sb[:, lo:hi], ps[:, lo:hi])
                    nc.sync.dma_start(yt_d[:, lo:hi], out_sb[:, lo:hi])

    nc.compile()
    return nc


def _get_module():
    if "nc" not in _MODULE_CACHE:
        _MODULE_CACHE["nc"] = _build_module()
    return _MODULE_CACHE["nc"]


def _prepare_inputs(x, cir_weights):
    x_np = _FP8E4 if MODE == "dr" else _BF16
    w_np = _FP8E4 if MODE == "dr" else _FP8E3

    xb = np.asarray(x, dtype=np.float32).reshape(I_BLOCKS, BLOCK)
    W = np.asarray(cir_weights, dtype=np.float32)

    # centered weights r = w - 0.5 in [-0.5, 0.5]; [b, j, o] fp8, contiguous
    WT = np.ascontiguousarray((W - np.float32(0.5)).astype(w_np).transpose(2, 1, 0))

    xx = xb.astype(x_np).reshape(JT_TILES, 128, BLOCK)  # [jt, j', c]

    in_maps = []
    for c in range(N_CORES):
        # Group order on core c: g = q*JT_TILES + jt with phase b = 16c+15-q,
        # so the on-chip window walk (src offset q+a) sees ascending q.
        # Host-side roll D_c makes the fixed kernel offset correct per core:
        #   xb2_c[j', jt, cc] = xb[jt*128+j', (cc + D_c) mod 128]
        D_c = (-(B_PER_CORE * c) - (B_PER_CORE - 1)) % BLOCK
        rolled = np.roll(xx, -D_c, axis=2)               # [jt, j', c]
        xb2 = np.concatenate([rolled, rolled], axis=2)[:, :, :XB2W]  # [jt, j', 144]
        xb2 = np.ascontiguousarray(xb2.transpose(1, 0, 2))  # [j', jt, 256]

        sub = WT[c * B_PER_CORE : (c + 1) * B_PER_CORE]  # [b_idx, j, o], b asc
        sub = sub[::-1]                                  # q = 15 - b_idx
        sub = sub.reshape(N_GROUPS, 128, O_BLOCKS)       # [g=(q,jt), j', o]
        wt = np.ascontiguousarray(sub.transpose(1, 0, 2))  # [j', g, o]

        in_maps.append({"xb2": xb2, "wt": wt})
    return in_maps


def kernel(x, cir_weights):
    from concourse.bass_utils import run_bass_kernel_spmd

    nc = _get_module()
    in_maps = _prepare_inputs(x, cir_weights)
    res = run_bass_kernel_spmd(nc, in_maps, core_ids=list(range(N_CORES)))

    yt = np.zeros((BLOCK, O_BLOCKS), dtype=np.float32)
    for r in res.results:
        yt += r["yt"]
    # rank-1 part of the centered-weight decomposition: w = 0.5 + r makes
    # y = R@x + 0.5*sum(x) in every output coordinate
    yt += np.float32(0.5 * np.sum(np.asarray(x, dtype=np.float64)))
    return np.ascontiguousarray(yt.T).reshape(O_BLOCKS * BLOCK)


# revision 15
# speedup vs baseline: 1.0575x; 1.0167x over previous
"""Block-circulant matvec (FFT linear layer) as dense TensorE matmuls on 8 TRN2 cores.

Math: the reference computes, per output block o,
    y[o, :] = sum_j IFFT(FFT(w[o,j]) * FFT(x[j])).real
which is a sum of circular convolutions:
    y[o, a] = sum_{j, b} w[o, j, b] * x[j, (a - b) mod 128]

Rewritten as matmuls: for each phase b and input-block tile jt (4 tiles of 128),
    YT[a, o] += XR(b,jt)[j', a]^T @ WT(b,jt)[j', o]
where XR(b,jt)[j', a] = x[jt*128+j', (a-b) mod 128] (rotated x tile, stationary)
and   WT(b,jt)[j', o] = w[o, jt*128+j', b]          (moving operand, N=512).

Sharding: the 128 phases b are split 16-per-core across 8 cores; each core
accumulates its 64 (b, jt) groups into one PSUM bank [128a x 512o] and writes a
partial YT. The host sums the 8 partials (no collective needed).

Precision: weights are centered (r = w - 0.5, uniform in [-0.5, 0.5]) and
shipped as fp8 e4m3 — halving HBM traffic vs bf16 (4.2 MiB/core, streamed at
the ~370-430 GB/s DMA floor).  x is also fp8 e4m3, and consecutive group
pairs fuse into DoubleRow matmuls (256-deep contraction, 2 fp8 MACs per cell
per cycle) so the 32 matmuls take only ~7 us and the kernel is DMA-bound.
Measured end-to-end quantization error is 0.87% (gate 2e-2).  The rank-1
part folds into one host-side scalar add after the gather (w = 0.5 + r makes
y = R@x + 0.5*sum(x)*ones).

On-chip details: the stationary operand of each matmul reads its rotated
window DIRECTLY from the tiny doubled-x buffer via an overlapping-window AP
(lhsT[j', t, a] = xb2[j', jt+t, q + a]) -- no materialized rotated copies, so
the Vector engine does no bulk work and the weight DMA stream keeps the SBUF
ports to itself.  The per-core phase offset is folded into a host-side roll
of the x buffer so the SPMD program is core-independent.  All input DMA rides
the sync-engine HWDGE ring in consumption order (x, then uniform 8-group
weight chunks whose ~64% PE duty cycle holds the HAM clock gate at 2.4 GHz);
N=128 warm-up matmuls cover the gate's ~3.4us ramp before the first chunk
lands, and a few filler matmuls after each chunk burst top up PE activity so
the gate never re-throttles mid-stream.  PSUM is evacuated in halves on the
Scalar and Vector engines in parallel, stored via both HWDGE rings.
"""

import numpy as np
import ml_dtypes

O_BLOCKS = 512
I_BLOCKS = 512
BLOCK = 128
N_CORES = 8
B_PER_CORE = BLOCK // N_CORES          # 16 phases per core
JT_TILES = I_BLOCKS // 128             # 4 contraction tiles
N_GROUPS = B_PER_CORE * JT_TILES       # 64 matmul groups per core
XB2W = 144  # doubled-x width: windows read [q, q+127], q<16 -> 143 max
CHUNK_GROUPS = (16, 16, 16, 8, 2, 2, 2, 2)  # all even: DoubleRow pairs never straddle
assert sum(CHUNK_GROUPS) == N_GROUPS
N_WARMUP_MM = 36   # ~3.8us of sustained PE busy: flips the HAM clock gate to
# 2.4 GHz just before the first real chunk's matmuls issue
N_FILLER_MM = 4    # cheap matmuls after each chunk burst keep PE duty high
# enough during DMA waits that the gate never re-throttles

MODE = "dr"  # "dr" (DoubleRow fp8e4) or "mixed" (bf16 x fp8e3)

_BF16 = ml_dtypes.bfloat16
_FP8E3 = ml_dtypes.float8_e3m4
_FP8E4 = ml_dtypes.float8_e4m3

_MODULE_CACHE = {}


def _build_module():
    import concourse.bass as bass
    import concourse.bacc as bacc
    import concourse.mybir as mybir
    from concourse import tile

    dr = MODE == "dr"
    x_dt = mybir.dt.float8e4 if dr else mybir.dt.bfloat16
    w_dt = mybir.dt.float8e4 if dr else mybir.dt.float8e3

    nc = bacc.Bacc(
        "TRN2",
        target_bir_lowering=False,
        debug=False,
        enable_asserts=False,
        enable_partition_id=False,
        num_devices=N_CORES,
    )

    xb2_d = nc.dram_tensor("xb2", [128, JT_TILES, XB2W], x_dt, kind="ExternalInput")
    wt_d = nc.dram_tensor("wt", [128, N_GROUPS, O_BLOCKS], w_dt, kind="ExternalInput")
    yt_d = nc.dram_tensor("yt", [BLOCK, O_BLOCKS], mybir.dt.float32, kind="ExternalOutput")

    with tile.TileContext(nc) as tc:
        with (
            tc.tile_pool(name="xbp", bufs=1) as xbp,
            tc.tile_pool(name="wtp", bufs=len(CHUNK_GROUPS)) as wtp,
            tc.tile_pool(name="psp", bufs=2, space="PSUM") as psp,
            tc.tile_pool(name="outp", bufs=1) as outp,
            tc.tile_pool(name="scrp", bufs=1) as scrp,
        ):
            scr = scrp.tile([128, 2 * BLOCK], mybir.dt.bfloat16)
            nc.gpsimd.memset(scr[:], 0.0)
            ps_warm = psp.tile([BLOCK, BLOCK], mybir.dt.float32)

            def scr_mm(n):
                for _ in range(n):
                    nc.tensor.matmul(
                        ps_warm[:], scr[:, :BLOCK], scr[:, BLOCK:],
                        start=True, stop=True, skip_group_check=True,
                    )

            scr_mm(N_WARMUP_MM)

            # all input DMA on the sync ring, in consumption order
            xb2_sb = xbp.tile([128, JT_TILES, XB2W], x_dt)
            nc.sync.dma_start(xb2_sb[:], xb2_d[:])
            xb2_ap = xb2_sb[:]

            def lhsT_window(m):
                # stationary operand for pair m: groups (2m, 2m+1), i.e.
                # q = m // 2, jt0 = 2*(m % 2); lhsT[p, t, a] = xb2[p, jt0+t, q+a]
                q, jt0 = m // 2, 2 * (m % 2)
                return bass.AP(
                    tensor=xb2_ap.tensor,
                    offset=xb2_ap.offset + jt0 * XB2W + q,
                    ap=[
                        xb2_ap.ap[0],        # partition (j')
                        [XB2W, 2],           # t: jt pair member
                        [1, BLOCK],          # a (overlapping window)
                    ],
                )

            def lhsT_single(g):
                q, jt = divmod(g, JT_TILES)
                return bass.AP(
                    tensor=xb2_ap.tensor,
                    offset=xb2_ap.offset + jt * XB2W + q,
                    ap=[xb2_ap.ap[0], [1, BLOCK]],
                )

            ps = psp.tile([BLOCK, O_BLOCKS], mybir.dt.float32)

            g0 = 0
            for ci, n_g in enumerate(CHUNK_GROUPS):
                wt_sb = wtp.tile([128, n_g, O_BLOCKS], w_dt, tag="wchunk")
                nc.sync.dma_start(wt_sb[:], wt_d[:, g0 : g0 + n_g, :])
                if dr:
                    for pi in range(n_g // 2):
                        g = g0 + 2 * pi
                        nc.tensor.matmul(
                            ps[:],
                            lhsT_window(g // 2),
                            wt_sb[:, 2 * pi : 2 * pi + 2, :],
                            start=(g == 0),
                            stop=(g == N_GROUPS - 2),
                            perf_mode=mybir.MatmulPerfMode.DoubleRow,
                            skip_group_check=True,
                        )
                else:
                    for gi in range(n_g):
                        g = g0 + gi
                        nc.tensor.matmul(
                            ps[:],
                            lhsT_single(g),
                            wt_sb[:, gi, :],
                            start=(g == 0),
                            stop=(g == N_GROUPS - 1),
                            skip_group_check=True,
                        )
                g0 += n_g
                if g0 < N_GROUPS:
                    scr_mm(N_FILLER_MM)

            # evacuate PSUM in quarters, alternating Scalar/Vector copies so
            # both run in parallel and the first store issues after only a
            # quarter-copy; stores alternate between the two HWDGE rings
            out_sb = outp.tile([BLOCK, O_BLOCKS], mybir.dt.float32)
            qo = O_BLOCKS // 4
            for k in range(4):
                lo, hi = k * qo, (k + 1) * qo
                nc.vector.tensor_copy(out_sb[:, lo:hi], ps[:, lo:hi])
                eng = nc.scalar if k % 2 == 0 else nc.sync
                eng.dma_start(yt_d[:, lo:hi], out_sb[:, lo:hi])

    nc.compile()
    return nc


def _get_module():
    if "nc" not in _MODULE_CACHE:
        _MODULE_CACHE["nc"] = _build_module()
    return _MODULE_CACHE["nc"]


def _prepare_inputs(x, cir_weights):
    x_np = _FP8E4 if MODE == "dr" else _BF16
    w_np = _FP8E4 if MODE == "dr" else _FP8E3

    xb = np.asarray(x, dtype=np.float32).reshape(I_BLOCKS, BLOCK)
    W = np.asarray(cir_weights, dtype=np.float32)

    # centered weights r = w - 0.5 in [-0.5, 0.5]; [b, j, o] fp8, contiguous
    WT = np.ascontiguousarray((W - np.float32(0.5)).astype(w_np).transpose(2, 1, 0))

    xx = xb.astype(x_np).reshape(JT_TILES, 128, BLOCK)  # [jt, j', c]

    in_maps = []
    for c in range(N_CORES):
        # Group order on core c: g = q*JT_TILES + jt with phase b = 16c+15-q,
        # so the on-chip window walk (src offset q+a) sees ascending q.
        # Host-side roll D_c makes the fixed kernel offset correct per core:
        #   xb2_c[j', jt, cc] = xb[jt*128+j', (cc + D_c) mod 128]
        D_c = (-(B_PER_CORE * c) - (B_PER_CORE - 1)) % BLOCK
        rolled = np.roll(xx, -D_c, axis=2)               # [jt, j', c]
        xb2 = np.concatenate([rolled, rolled], axis=2)[:, :, :XB2W]  # [jt, j', 144]
        xb2 = np.ascontiguousarray(xb2.transpose(1, 0, 2))  # [j', jt, 256]

        sub = WT[c * B_PER_CORE : (c + 1) * B_PER_CORE]  # [b_idx, j, o], b asc
        sub = sub[::-1]                                  # q = 15 - b_idx
        sub = sub.reshape(N_GROUPS, 128, O_BLOCKS)       # [g=(q,jt), j', o]
        wt = np.ascontiguousarray(sub.transpose(1, 0, 2))  # [j', g, o]

        in_maps.append({"xb2": xb2, "wt": wt})
    return in_maps


def kernel(x, cir_weights):
    from concourse.bass_utils import run_bass_kernel_spmd

    nc = _get_module()
    in_maps = _prepare_inputs(x, cir_weights)
    res = run_bass_kernel_spmd(nc, in_maps, core_ids=list(range(N_CORES)))

    yt = np.zeros((BLOCK, O_BLOCKS), dtype=np.float32)
    for r in res.results:
        yt += r["yt"]
    # rank-1 part of the centered-weight decomposition: w = 0.5 + r makes
    # y = R@x + 0.5*sum(x) in every output coordinate
    yt += np.float32(0.5 * np.sum(np.asarray(x, dtype=np.float64)))
    return np.ascontiguousarray(yt.T).reshape(O_BLOCKS * BLOCK)
